# revision 57
# baseline (speedup 1.0000x reference)
"""Trainium2 Bass kernel for nn_MetaLinear3 (per-token rank-1 meta-linear).

Reference math (per token x in R^D, D=512):
    w1 = W_in @ x ; w2 = W_out @ x ; br = W_b @ x
    w  = outer(w2, w1), layer-normed over the last dim, then y = w @ x + LN(br)

The rank-1 structure survives the layernorm:
    LN_row_i(w)[j] = w2[i] * (w1[j] - mean(w1)) / sqrt(w2[i]^2 * var(w1) + EPS)
so with t = sum_j (w1[j] - mean(w1)) * x[j] (a scalar per token):
    y[i] = w2[i] * t / sqrt(w2[i]^2 * var(w1) + EPS)
           + (br[i] - mean(br)) / sqrt(var(br) + EPS)

This reduces the [B,S,D,D] intermediate to 3 matmuls + per-token reductions.

Sharding: data parallel across 8 NeuronCores. 1024 tokens / 8 = 128 tokens
per core = the SBUF partition count. Weights replicated per core.
"""

import numpy as np

import concourse.mybir as mybir
from concourse import bacc
from concourse.bass_utils import run_bass_kernel_spmd
from concourse.tile import TileContext

F32 = mybir.dt.float32
EPS = 1e-5
B, S, D = 2, 512, 512
P = 128          # SBUF partitions = tokens per core
KC = D // P      # contraction chunks
N_CORES = 8


# Per-chunk SBUF blob layouts (partition dim = 128 d-rows of chunk k), all
# bf16, one DMA per chunk per phase:
#   tr_k: [ x_hi 128 | x_lo 128 | Win_hi 512 ]  -> w1 (2-term)
#   tb_k: [ Wb_hi 512 ]                         -> bb (single term)
#   th_k: [ W2_hi 512 | W2_lo 512 ]             -> w2 (3-term, two
#          half-column PSUM groups)
# x_hi/x_lo are shared by all three matmul groups, and the token-major x
# needed for the t-reduction is rebuilt on-chip by PE-transposing x_hi/x_lo
# (saves the x_tok DMA entirely). An identity tile for the transposes is the
# only other input.
#
# Precision: w2 uses 3 bf16 terms (hi*hi + hi*lo + lo*hi, abs err ~1e-5)
# because y's w2-path slope near w2=0 is t/sqrt(EPS) (~4e3), which amplifies
# absolute w2 error ~3.8e3x (f32r's 4e-4 abs err fails here: 3.7e-2 measured).
# w1 takes 2 terms and bb a single term: their errors enter y through smooth,
# unamplified paths (measured on HW: whole-kernel error stays ~1.6e-3).
#
# DMA phase order: tr (w1 stats + x transpose feed everything) -> tb (bb/bn
# chain) -> th (w2 tail is the only post-last-byte work).
TRF = 2 * P + D         # tr columns (w1 is 2-term: only Win_hi shipped)
TBF = D                 # tb columns
THF = 2 * D             # th columns
BF16 = mybir.dt.bfloat16

# USE_ARS=True fuses sqrt+reciprocal into one ACT Abs_reciprocal_sqrt op
# (HW-validated: error-neutral). CoreSim lacks ARS, so simulation harnesses
# flip this to False.
USE_ARS = True


def build_nc():
    nc = bacc.Bacc("TRN2", target_bir_lowering=False, debug=False,
                   num_devices=N_CORES)
    tr_d = nc.declare_dram_parameter("blob_r", [P, KC, TRF], BF16, isOutput=False)
    tb_d = nc.declare_dram_parameter("blob_b", [P, KC, TBF], BF16, isOutput=False)
    th_d = nc.declare_dram_parameter("blob_h", [P, KC, THF], BF16, isOutput=False)
    y_d = nc.declare_dram_parameter("y", [P, D], F32, isOutput=True)

    from concourse.tile_rust import add_dep_helper

    with TileContext(nc) as tc:
        with (
            tc.tile_pool(name="main", bufs=1) as pool,
            tc.tile_pool(name="psum", bufs=1, space="PSUM") as pp,
        ):
            stat = lambda n: pool.tile([P, 1], F32, name=n)
            epsb = stat("epsb")
            nc.vector.memset(epsb[:], EPS)
            # Warm the ACT function table containing Sqrt up front (a cold
            # LoadActFuncSet is ~1.3us and would otherwise land mid-tail).
            warm = stat("warm")
            warm2 = stat("warm2")
            warm_fn = (mybir.ActivationFunctionType.Abs_reciprocal_sqrt if USE_ARS
                       else mybir.ActivationFunctionType.Sqrt)
            nc.scalar.activation(warm[:], epsb[:], warm_fn, bias=epsb[:])
            # PE HAM warmup: ~3us of junk matmuls while the first DMAs are in
            # flight releases the clock gate, so the real matmuls run at full
            # rate instead of the half-rate cold p-state.
            junk = pool.tile([P, D], BF16)
            nc.gpsimd.memset(junk[:], 0.0)

            # Pair up chunk DMAs: the HWDGE descriptor slot is ~625 ns per
            # dma_start, so chunk-sized transfers (360-550 ns) would be
            # descriptor-bound. Two chunks per DMA keeps the queue
            # transfer-bound while still pipelining PE behind the DMA.
            # Identity for the PE transposes, built on-chip: iota gives
            # (col - partition) per element, is_equal 0 -> 1.0 on the diagonal.
            ident = pool.tile([P, P], BF16)
            identi = pool.tile([P, P], mybir.dt.int16)
            nc.gpsimd.iota(identi[:], pattern=[[1, P]], base=0,
                           channel_multiplier=-1)
            nc.vector.tensor_scalar(ident[:], identi[:], 0, None,
                                    op0=mybir.AluOpType.is_equal)
            tr01 = pool.tile([P, 2, TRF], BF16)
            tr23 = pool.tile([P, 2, TRF], BF16)
            tball = pool.tile([P, KC, TBF], BF16)
            nc.sync.dma_start(tr01[:], tr_d[:, 0:2, :])
            nc.sync.dma_start(tr23[:], tr_d[:, 2:4, :])
            nc.sync.dma_start(tball[:], tb_d[:])
            # th per chunk: after the last chunk's bytes land, only that
            # chunk's 6 quarter-matmuls remain before the w2 tail starts.
            tht = [pool.tile([P, THF], BF16, name=f"tht{k}") for k in range(KC)]
            for k in range(KC):
                nc.sync.dma_start(tht[k][:], th_d[:, k, :])

            tr = [tr01[:, 0, :], tr01[:, 1, :], tr23[:, 0, :], tr23[:, 1, :]]
            tb = [tball[:, k, :] for k in range(KC)]
            th = [tht[k][:] for k in range(KC)]
            xh = [tr[k][:, 0:P] for k in range(KC)]
            xl = [tr[k][:, P:2 * P] for k in range(KC)]

            w1 = pp.tile([P, D], F32)
            bb = pp.tile([P, D], F32)
            NQ = 2
            Q = D // NQ
            w2q = [pp.tile([P, Q], F32, name=f"w2q{q}") for q in range(NQ)]
            # Token-major x rebuilt on-chip: transpose x_hi/x_lo chunks.
            xhT = pp.tile([P, D], BF16)
            xlT = pp.tile([P, D], BF16)
            # PE HAM warmup matmuls write into bb's bank (safe: bb's first
            # real matmul has start=True, which re-zeroes it) so all 8 PSUM
            # banks stay available for real tiles.
            for _ in range(8):
                nc.tensor.matmul(bb[:], junk[:, 0:P], junk[:, 0:D],
                                 start=True, stop=True)
            for k in range(KC):
                st, sp = (k == 0), (k == KC - 1)
                winh = tr[k][:, 2 * P:2 * P + D]
                nc.tensor.matmul(w1[:], xh[k], winh, start=st, stop=False)
                nc.tensor.matmul(w1[:], xl[k], winh, start=False, stop=sp)
                nc.tensor.transpose(xhT[:, k * P:(k + 1) * P], xh[k], ident[:])
                nc.tensor.transpose(xlT[:, k * P:(k + 1) * P], xl[k], ident[:])
            for k in range(KC):
                nc.tensor.matmul(bb[:], xh[k], tb[k][:, 0:D],
                                 start=(k == 0), stop=(k == KC - 1))
            for k in range(KC):
                st, sp = (k == 0), (k == KC - 1)
                w2h = th[k][:, 0:D]
                w2l = th[k][:, D:2 * D]
                for q in range(NQ):
                    cols = slice(q * Q, (q + 1) * Q)
                    nc.tensor.matmul(w2q[q][:], xh[k], w2h[:, cols], start=st, stop=False)
                    nc.tensor.matmul(w2q[q][:], xh[k], w2l[:, cols], start=False, stop=False)
                    nc.tensor.matmul(w2q[q][:], xl[k], w2h[:, cols], start=False, stop=sp)

            # xtok = x_hi.T + x_lo.T in f32 (ACT copy from PSUM, DVE add).
            xtok1 = pool.tile([P, D], F32)
            xtok = pool.tile([P, D], F32)
            nc.scalar.copy(xtok1[:], xhT[:])
            nc.vector.scalar_tensor_tensor(xtok[:], xlT[:], 1.0, xtok1[:],
                                           op0=mybir.AluOpType.mult,
                                           op1=mybir.AluOpType.add)

            scr1 = pool.tile([P, D], F32)
            scr2 = pool.tile([P, D], F32)

            # ---- w1 stats (tr phase; large slack) ----
            sum1, sumsq1 = stat("sum1"), stat("sumsq1")
            m1, msq, var1, t = stat("m1"), stat("msq"), stat("var1"), stat("t")
            nc.vector.reduce_sum(sum1[:], w1[:], axis=mybir.AxisListType.X)
            nc.scalar.activation(scr1[:], w1[:], mybir.ActivationFunctionType.Square,
                                 accum_out=sumsq1[:])
            nc.vector.tensor_scalar_mul(m1[:], sum1[:], 1.0 / D)
            nc.vector.tensor_mul(msq[:], m1[:], m1[:])
            # var1 = sumsq1/D - m1^2
            nc.vector.scalar_tensor_tensor(var1[:], sumsq1[:], 1.0 / D, msq[:],
                                           op0=mybir.AluOpType.mult,
                                           op1=mybir.AluOpType.subtract)
            # t = sum((w1 - m1) * x)
            nc.vector.scalar_tensor_tensor(scr1[:], w1[:], m1[:], xtok[:],
                                           op0=mybir.AluOpType.subtract,
                                           op1=mybir.AluOpType.mult,
                                           accum_out=t[:])

            # ---- b path (tb phase): bn = (bb - mb) * rsqrt(vb + EPS) ----
            sumb, sumsqb = stat("sumb"), stat("sumsqb")
            mb, msqb, vb, rb = stat("mb"), stat("msqb"), stat("vb"), stat("rb")
            nc.vector.reduce_sum(sumb[:], bb[:], axis=mybir.AxisListType.X)
            nc.scalar.activation(scr2[:], bb[:], mybir.ActivationFunctionType.Square,
                                 accum_out=sumsqb[:])
            nc.vector.tensor_scalar_mul(mb[:], sumb[:], 1.0 / D)
            nc.vector.tensor_mul(msqb[:], mb[:], mb[:])
            nc.vector.scalar_tensor_tensor(vb[:], sumsqb[:], 1.0 / D, msqb[:],
                                           op0=mybir.AluOpType.mult,
                                           op1=mybir.AluOpType.subtract)
            if USE_ARS:
                # rb = 1/sqrt(vb + EPS) in one ACT op (same table as warm)
                nc.scalar.activation(
                    rb[:], vb[:],
                    mybir.ActivationFunctionType.Abs_reciprocal_sqrt,
                    bias=epsb[:])
            else:
                nc.scalar.activation(vb[:], vb[:],
                                     mybir.ActivationFunctionType.Sqrt,
                                     bias=epsb[:])
                nc.vector.reciprocal(rb[:], vb[:])
            bn = pool.tile([P, D], F32)
            for q in range(2):
                cols = slice(q * (D // 2), (q + 1) * (D // 2))
                nc.vector.tensor_scalar(bn[:, cols], bb[:, cols], mb[:], rb[:],
                                        op0=mybir.AluOpType.subtract,
                                        op1=mybir.AluOpType.mult)

            # ---- per half (th phase): y = (w2*t)/sqrt(w2^2*var1+EPS) + bn
            y = pool.tile([P, D], F32)
            i_prev = None
            for q in range(NQ):
                cols = slice(q * Q, (q + 1) * Q)
                sq2 = pool.tile([P, Q], F32, name=f"sq2_{q}")
                recip = pool.tile([P, Q], F32, name=f"recip_{q}")
                num = pool.tile([P, Q], F32, name=f"num_{q}")
                i_sq2 = nc.scalar.square(sq2[:], w2q[q][:])
                if i_prev is not None:
                    add_dep_helper(i_sq2.ins, i_prev.ins, sync=False,
                                   reason="ACT order: halves in sequence")
                if USE_ARS:
                    # tiny spacer between the RAW pair (ars reads sq2 written
                    # by the immediately preceding ACT op) absorbs part of
                    # the ~220ns ACT pipeline stall
                    i_sp = nc.scalar.copy(warm[:], epsb[:])
                    add_dep_helper(i_sp.ins, i_sq2.ins, sync=False,
                                   reason="ACT spacer after sq2")
                    i_prev = nc.scalar.activation(
                        recip[:], sq2[:],
                        mybir.ActivationFunctionType.Abs_reciprocal_sqrt,
                        bias=epsb[:], scale=var1[:])
                    add_dep_helper(i_prev.ins, i_sp.ins, sync=False,
                                   reason="ACT ars after spacer")
                else:
                    den = pool.tile([P, Q], F32, name=f"den_{q}")
                    i_prev = nc.scalar.activation(
                        den[:], sq2[:], mybir.ActivationFunctionType.Sqrt,
                        bias=epsb[:], scale=var1[:])
                    nc.vector.reciprocal(recip[:], den[:])
                i_num = nc.vector.scalar_tensor_tensor(num[:], w2q[q][:], t[:],
                                                       recip[:],
                                                       op0=mybir.AluOpType.mult,
                                                       op1=mybir.AluOpType.mult)
                # tiny DVE spacer absorbs the RAW pipeline stall between
                # num and the y add that reads it
                i_dsp = nc.vector.memset(warm2[:], 0.0)
                add_dep_helper(i_dsp.ins, i_num.ins, sync=False,
                               reason="DVE spacer after num")
                i_y = nc.vector.tensor_add(y[:, cols], num[:], bn[:, cols])
                add_dep_helper(i_y.ins, i_dsp.ins, sync=False,
                               reason="DVE y after spacer")
                # half 0's DMA issued from the idle ACT queue so SP's
                # descriptor slot is free the moment half 1's data lands.
                eng = nc.scalar if q == 0 else nc.sync
                eng.dma_start(y_d[:, cols], y[:, cols])
    nc.compile()
    return nc


def _chunk_dmajor(a_T, free):
    """[512, free] d-major -> [128, 4, free] chunked layout."""
    return np.ascontiguousarray(a_T.reshape(KC, P, free).transpose(1, 0, 2))


def _hi_lo(a):
    import ml_dtypes
    hi = a.astype(ml_dtypes.bfloat16)
    lo = (a - hi.astype(np.float32)).astype(ml_dtypes.bfloat16)
    return hi, lo


def make_in_maps(x, W_in, W_out, W_b):
    import ml_dtypes
    toks = np.ascontiguousarray(np.asarray(x).reshape(-1, D).astype(np.float32, copy=False))
    per = toks.shape[0] // N_CORES
    assert per == P
    winh = _chunk_dmajor(np.ascontiguousarray(np.asarray(W_in).T), D).astype(ml_dtypes.bfloat16)
    w2h, w2l = _hi_lo(_chunk_dmajor(np.ascontiguousarray(np.asarray(W_out).T), D))
    wbh = _chunk_dmajor(np.ascontiguousarray(np.asarray(W_b).T), D).astype(ml_dtypes.bfloat16)
    blob_h = np.ascontiguousarray(np.concatenate([w2h, w2l], axis=2))
    in_maps = []
    for c in range(N_CORES):
        xs = toks[c * per:(c + 1) * per]
        xTc = _chunk_dmajor(np.ascontiguousarray(xs.T), P)
        xhc, xlc = _hi_lo(xTc)
        blob_r = np.ascontiguousarray(np.concatenate([xhc, xlc, winh], axis=2))
        in_maps.append({"blob_r": blob_r, "blob_b": wbh, "blob_h": blob_h})
    return in_maps


_NC = None


def _get_nc():
    global _NC
    if _NC is None:
        _NC = build_nc()
    return _NC


def kernel(x, W_in, W_out, W_b):
    nc = _get_nc()
    in_maps = make_in_maps(x, W_in, W_out, W_b)
    res = run_bass_kernel_spmd(nc, in_maps, core_ids=list(range(N_CORES)))
    ys = np.concatenate([r["y"] for r in res.results], axis=0)
    return ys.reshape(B, S, D).astype(np.float32, copy=False)


def profile_exec_ns(x=None, W_in=None, W_out=None, W_b=None):
    """Per-core kernel duration from the instruction-level timeline simulator
    (the HW-calibrated cost model). Direct NTFF/neuron-profile capture is
    unavailable through this axon relay, and dispatch overhead (~100 ms)
    swamps wall-clock timing, so this is the best available HW-time proxy.
    The kernel is SPMD-identical per core, so core 0's timeline = all cores.
    """
    from concourse.timeline_sim import TimelineSim
    nc = build_nc()
    return int(TimelineSim(nc, trace=False).simulate())
